# revision 8
# baseline (speedup 1.0000x reference)
"""Trainium2 Bass kernel for nn_NNSK (gnn_message_passing, memory regime).

Strategy
--------
Per-edge math:  out[e, r] = a1[b_e, r] * (r0[b_e]/rij_e)^(1 + a2[b_e, r]) * fcut(rij_e)
with only 4 bond types b and 28 output quantities (14 hopping + 14 overlap).

Key transform: for each bond type b, the map r -> exp(c1[b,r] * L) with
L = ln(r0/rij) and c1 = 1+a2 is, to ~1e-8, a rank-4 function of the edge:
we Chebyshev-interpolate g(c) = exp(c*L) in the exponent c with 4 nodes c_k,
so   out[r, e] = sum_k V[k, r] * U_k[e],
     U_k[e]    = exp(c_k * L_e - sp_e),   sp = ln(1 + exp((rij-6)/0.2))  (-> fcut)
     V[k, r]   = a1[b, r] * lagrange_k(c1[b, r]).
That turns the gather+powerlaw into a K=16 (4 sub-chunks x 4 basis) fp32r
matmul with constant per-type weights, streaming 512 edges/instruction.

Host side: shard edges 8 ways, sort each core's edges by bond type (run
lengths padded identically across cores so one SPMD program serves all 8),
device computes per-edge transcendentals on ACT (Ln/Exp set only), builds the
rank-4 basis, matmuls on PE, evacuates PSUM via ACT/DVE, DMAs out.
Nodes: out = T0 + atom_type*(T1-T0), tiny.
"""
import os
import numpy as np
from contextlib import ExitStack

import concourse.bass as bass
import concourse.bacc as bacc
import concourse.tile as tile
from concourse import mybir
from concourse.bass_utils import run_bass_kernel_spmd

# ---- problem constants (hardcoded; kernel.py must be self-contained) ----
N_EDGES = 2_000_000
N_NODES = 100_000
N_CORES = 8
RS, WCUT = 6.0, 0.2
REFL = np.array([0, 2, 1, 3])
SYM_IDX = np.array([0, 4, 5, 11, 12, 13])
BOND_TO_PAIR = np.array([[14, 14], [14, 8], [8, 14], [8, 8]])
Z_OF_TYPE = np.array([14, 8])

FT = 512            # edges per sub-chunk (= matmul N, one PSUM bank)
GROUP_E = 4 * FT    # edges per macro-group (one matmul, 16 rhs rows)
TILE_E = 8 * GROUP_E  # edges per B-tile ([128, 512] sbuf tile)

F32 = mybir.dt.float32
F32R = mybir.dt.float32r
AF = mybir.ActivationFunctionType
ALU = mybir.AluOpType


def _symmetrize(p):
    out = p.copy()
    out[:, SYM_IDX, :] = 0.5 * (p[:, SYM_IDX, :] + p[REFL][:, SYM_IDX, :])
    return out


def _prepare_tables(hopping_param, overlap_param, bond_length_list):
    hp = _symmetrize(np.asarray(hopping_param, np.float64))
    op = _symmetrize(np.asarray(overlap_param, np.float64))
    sym_mask = np.zeros(14); sym_mask[SYM_IDX] = 1.0
    same_elem = (BOND_TO_PAIR[:, 0] == BOND_TO_PAIR[:, 1]).astype(np.float64)
    a1h = hp[:, :, 0]
    c1h = 1.0 + np.abs(hp[:, :, 1])
    a1o = op[:, :, 0] + same_elem[:, None] * sym_mask[None, :]
    c1o = 1.0 + np.abs(op[:, :, 1])
    bll = np.asarray(bond_length_list, np.float64)
    r0 = 0.5 * (bll[BOND_TO_PAIR[:, 0] - 1] + bll[BOND_TO_PAIR[:, 1] - 1])
    lnr0 = np.log(r0)

    V16 = np.zeros((4, 16, 112), np.float32)
    cnodes = np.zeros((4, 4), np.float64)
    for b in range(4):
        cv = np.concatenate([c1h[b], c1o[b]])
        a1 = np.concatenate([a1h[b], a1o[b]])
        lo, hi = cv.min(), cv.max()
        if hi - lo < 1e-6:
            lo, hi = lo - 1e-3, hi + 1e-3
        kk = np.arange(4)
        cn = (lo + hi) / 2 + (hi - lo) / 2 * np.cos((2 * kk + 1) / 8 * np.pi)
        cnodes[b] = cn
        ell = np.ones((4, 28))
        for k in range(4):
            for j in range(4):
                if j != k:
                    ell[k] *= (cv - cn[j]) / (cn[k] - cn[j])
        Vb = (ell * a1[None, :]).astype(np.float32)  # [4, 28]
        for q in range(4):
            V16[b, 4 * q:4 * q + 4, 28 * q:28 * q + 28] = Vb
    # [b, parity] -> [32, 112] with the 16-row block at rows 16*parity;
    # the zero half masks out the other macro-group sharing the row-group.
    V32 = np.zeros((4, 2, 32, 112), np.float32)
    for b in range(4):
        V32[b, 0, :16] = V16[b]
        V32[b, 1, 16:] = V16[b]
    return V32, cnodes, lnr0


def _register_const(nc, value):
    t = nc.alloc_sbuf_tensor(f"const-float32-{value}", [128, 1], F32)
    nc.gpsimd.memset(t.ap(), value)
    nc.const_aps.aps[(F32, float(value))] = t.ap()


def _build_device(Tt, gtype, node_t0, node_d):
    nc = bacc.Bacc("TRN2", target_bir_lowering=False, debug=False,
                   num_devices=N_CORES)
    _register_const(nc, -30.0)
    rij = nc.dram_tensor("rij", [Tt, 128, FT], F32, kind="ExternalInput").ap()
    vw = nc.dram_tensor("vw", [128, 896], F32, kind="ExternalInput").ap()
    cneg = nc.dram_tensor("cneg", [128, Tt], F32, kind="ExternalInput").ap()
    dall = nc.dram_tensor("dall", [128, Tt], F32, kind="ExternalInput").ap()
    tyn = nc.dram_tensor("tyn", [128, 98], F32, kind="ExternalInput").ap()
    eout = nc.dram_tensor("eout", [Tt, 8, 112, FT], F32, kind="ExternalOutput").ap()
    nout = nc.dram_tensor("nout", [128, 294], F32, kind="ExternalOutput").ap()

    with tile.TileContext(nc) as tc, ExitStack() as ctx:
        consts = ctx.enter_context(tc.tile_pool(name="consts", bufs=1))
        inp = ctx.enter_context(tc.tile_pool(name="inp", bufs=3))
        mid = ctx.enter_context(tc.tile_pool(name="mid", bufs=2))
        stg = ctx.enter_context(tc.tile_pool(name="stg", bufs=4))
        psum = ctx.enter_context(
            tc.tile_pool(name="psum", bufs=4, space="PSUM"))

        vw_sb = consts.tile([128, 896], F32)
        nc.sync.dma_start(vw_sb[:], vw)
        cneg_sb = consts.tile([128, Tt], F32)
        nc.sync.dma_start(cneg_sb[:], cneg)
        dall_sb = consts.tile([128, Tt], F32)
        nc.sync.dma_start(dall_sb[:], dall)

        # ---- node part (tiny) ----
        tyn_sb = consts.tile([128, 98], F32)
        nc.sync.dma_start(tyn_sb[:], tyn)
        nout_sb = consts.tile([128, 294], F32)
        nv = nout_sb[:].rearrange("p (f j) -> p f j", j=3)
        for j in range(3):
            nc.vector.tensor_scalar(nv[:, :, j], tyn_sb[:],
                                    float(node_d[j]), float(node_t0[j]),
                                    ALU.mult, ALU.add)
        nc.sync.dma_start(nout, nout_sb[:])

        # ---- edge tiles ----
        for t in range(Tt):
            r = inp.tile([128, FT], F32, tag="r")
            nc.sync.dma_start(r[:], rij[t])
            lnr = mid.tile([128, FT], F32, tag="lnr")
            nc.scalar.activation(lnr[:], r[:], AF.Ln)
            s = mid.tile([128, FT], F32, tag="s")
            nc.scalar.activation(s[:], r[:], AF.Exp, bias=-30.0, scale=5.0)
            sp = mid.tile([128, FT], F32, tag="sp")
            nc.scalar.activation(sp[:], s[:], AF.Ln, bias=1.0, scale=1.0)
            # y = -c*ln(rij) - sp ;  U = exp(y + c*ln(r0))
            y = mid.tile([128, FT], F32, tag="y")
            nc.vector.scalar_tensor_tensor(
                y[:], lnr[:], cneg_sb[:, t:t + 1], sp[:],
                ALU.mult, ALU.subtract)
            u = mid.tile([128, FT], F32, tag="u")
            nc.scalar.activation(u[:], y[:], AF.Exp,
                                 bias=dall_sb[:, t:t + 1], scale=1.0)
            ur = u[:]
            for g in range(8):
                b = int(gtype[t * 8 + g])
                R, par = g // 2, g % 2
                v = b * 2 + par
                p = psum.tile([112, FT], F32, tag="p")
                nc.tensor.matmul(
                    p[:],
                    vw_sb[32 * R:32 * (R + 1), 112 * v:112 * (v + 1)],
                    ur[32 * R:32 * (R + 1), :],
                    start=True, stop=True,
                    tile_position=(32 * R, 0))
                o = stg.tile([112, FT], F32, tag="o")
                if g % 2 == 0:
                    nc.scalar.copy(o[:], p[:])
                else:
                    nc.vector.tensor_copy(o[:], p[:])
                nc.sync.dma_start(eout[t, g], o[:])
    nc.compile()
    return nc


def kernel(hopping_param, overlap_param, onsite_param, onsite_E_base,
           bond_length_list, edge_length, edge_type, atom_type):
    V32, cnodes, lnr0 = _prepare_tables(hopping_param, overlap_param,
                                        bond_length_list)
    el = np.asarray(edge_length, np.float32)
    ety = np.asarray(edge_type).astype(np.int64)
    aty = np.asarray(atom_type).astype(np.int64)

    Ec = N_EDGES // N_CORES
    counts = np.zeros((N_CORES, 4), np.int64)
    orders = []
    for c in range(N_CORES):
        t = ety[c * Ec:(c + 1) * Ec]
        orders.append(np.argsort(t, kind="stable"))
        counts[c] = np.bincount(t, minlength=4)

    run_len = (np.ceil(counts.max(axis=0) / GROUP_E) * GROUP_E).astype(np.int64)
    total = int(run_len.sum())
    Tt = int(np.ceil(total / TILE_E))
    run_len[3] += Tt * TILE_E - total
    total_pad = Tt * TILE_E
    offs = np.concatenate([[0], np.cumsum(run_len)])[:4]

    # group -> bond type (identical across cores)
    G = total_pad // GROUP_E
    bounds = np.cumsum(run_len)
    gtype = np.searchsorted(bounds, np.arange(G) * GROUP_E, side="right")

    # per-core sorted/padded rij + slot bookkeeping for the un-permute
    rij_dev = np.full((N_CORES, total_pad), 2.0, np.float32)
    slots_all = []
    for c in range(N_CORES):
        order = orders[c]
        slots = np.empty(Ec, np.int64)
        pos = 0
        for b in range(4):
            n = int(counts[c, b])
            slots[pos:pos + n] = offs[b] + np.arange(n)
            pos += n
        rij_dev[c, slots] = el[c * Ec:(c + 1) * Ec][order]
        slots_all.append(slots)

    # replicate x4: row of tile t = g*16 + q*4 + k  (k = basis index)
    rep = rij_dev.reshape(N_CORES, Tt, 8, 4, 1, FT)
    rep = np.broadcast_to(rep, (N_CORES, Tt, 8, 4, 4, FT))
    rep = np.ascontiguousarray(rep).reshape(N_CORES, Tt, 128, FT)

    # per-(row, tile) basis constants
    rows = np.arange(128)
    ks = rows % 4
    c_all = np.zeros((128, Tt), np.float32)
    d_all = np.zeros((128, Tt), np.float32)
    for t in range(Tt):
        b_row = gtype[t * 8 + rows // 16]
        cv = cnodes[b_row, ks]
        c_all[:, t] = cv
        d_all[:, t] = cv * lnr0[b_row]
    cneg = -c_all

    # nodes
    node_tab = (np.asarray(onsite_E_base, np.float64)[Z_OF_TYPE - 1]
                + np.asarray(onsite_param, np.float64)[:, :, 0])  # [2, 3]
    node_t0 = node_tab[0]
    node_d = node_tab[1] - node_tab[0]
    Ncn = N_NODES // N_CORES
    NP = 128 * 98
    tyn = np.zeros((N_CORES, NP), np.float32)
    for c in range(N_CORES):
        tyn[c, :Ncn] = aty[c * Ncn:(c + 1) * Ncn]
    tyn = tyn.reshape(N_CORES, 128, 98)

    # [32, 8*112]: variant v = b*2 + parity along free; replicated x4 along
    # partitions so lhsT can start at any 32-row group (walrus requires
    # fmap/weight same start partition).
    vw2d = np.ascontiguousarray(
        V32.reshape(8, 32, 112).transpose(1, 0, 2).reshape(32, 896))
    vw2d = np.tile(vw2d, (4, 1))
    nc = _build_device(Tt, gtype, node_t0, node_d)
    in_maps = [{
        "rij": rep[c],
        "vw": vw2d,
        "cneg": cneg,
        "dall": d_all,
        "tyn": tyn[c],
    } for c in range(N_CORES)]
    trace = bool(int(os.environ.get("BASS_KERNEL_TRACE", "0")))
    res = run_bass_kernel_spmd(nc, in_maps, core_ids=list(range(N_CORES)),
                               trace=trace)
    if trace and res.exec_time_ns is not None:
        print(f"HW exec time: {res.exec_time_ns} ns "
              f"(mean {res.mean_exec_time_ns} ns, "
              f"slowest core {res.max_exec_time_core_id})")
        if res.instructions_and_trace:
            print("trace:", res.instructions_and_trace[1])

    ef = np.empty((N_EDGES, 14), np.float32)
    eo = np.empty((N_EDGES, 14), np.float32)
    nf = np.empty((N_NODES, 3), np.float32)
    for c in range(N_CORES):
        out = res.results[c]["eout"]  # [Tt, 8, 112, FT]
        arr = out.reshape(G, 4, 28, FT).transpose(0, 1, 3, 2)
        arr = arr.reshape(total_pad, 28)
        vals = arr[slots_all[c]]
        base = c * Ec
        ef[base + orders[c]] = vals[:, :14]
        eo[base + orders[c]] = vals[:, 14:]
        nodev = res.results[c]["nout"].reshape(128, 98, 3).reshape(NP, 3)
        nf[c * Ncn:(c + 1) * Ncn] = nodev[:Ncn]
    return ef, eo, nf
